# revision 11
# baseline (speedup 1.0000x reference)
"""Trainium2 Bass kernel: 12-head self-attention (B=8, N=1024, D=768).

Sharding: data-parallel over batch — one batch element per NeuronCore,
weights replicated on all 8 cores, no collectives.

Per-core dataflow (all matmuls bf16 operands, fp32 PSUM accumulation):
  xT [768,1024] (host-pretransposed, bf16)
  qkT[t] = W_qk[:,t-chunk].T @ xT          (feature-major q/k, 12 chunks)
  v[mt]  = xT[:,mt-chunk].T @ W_v          (token-major v, + ones column
                                            per head for softmax row sums)
  per head h:
    S^T[mt] = kT_h[:,mt].T @ qT_h          ([keys,queries], K=64; even/odd
                                            heads pack PE row groups 0/64)
    P^T[mt] = exp(scale * S^T[mt])         (ACT, no max-subtraction: scores
                                            are ~N(0,1), exp is safe in f32)
    outT   += v'_h[mt].T @ P^T[mt]         ([65,1024]; row 64 = row sums)
    attn_T_h = outT[0:64] * bcast(1/outT[64])
  out[nt] = attn_T[:,nt].T @ W_p + b       (+ bias via broadcast add)
"""

from contextlib import ExitStack

import numpy as np
import ml_dtypes

import concourse.bacc as bacc
import concourse.bass as bass
import concourse.mybir as mybir
import concourse.tile as tile
from concourse.bass_utils import run_bass_kernel_spmd

B, N, D = 8, 1024, 768
H, HD = 12, 64
SCALE = HD ** -0.5
KC = D // 128          # 6 contraction chunks of 128
NT = N // 128          # 8 token tiles of 128
VW = 128               # per-head v slot: col 0 = ones, cols 64..127 = v data
F32 = mybir.dt.float32
BF16 = mybir.dt.bfloat16
NCORES = 8

_CACHE = {}


def _build_nc():
    nc = bacc.Bacc(None, target_bir_lowering=False)
    xT = nc.dram_tensor("xT", [D, N], BF16, kind="ExternalInput")
    w_qk = nc.dram_tensor("w_qk", [D, 2 * D], BF16, kind="ExternalInput")
    w_v = nc.dram_tensor("w_v", [D, D], BF16, kind="ExternalInput")
    w_p = nc.dram_tensor("w_p", [D, D], BF16, kind="ExternalInput")
    bias = nc.dram_tensor("bias", [1, D], F32, kind="ExternalInput")
    out = nc.dram_tensor("out", [N, D], F32, kind="ExternalOutput")

    with ExitStack() as ctx:
        tc = ctx.enter_context(tile.TileContext(nc))
        const = ctx.enter_context(tc.tile_pool(name="const", bufs=1))
        work = ctx.enter_context(tc.tile_pool(name="work", bufs=2))
        psA = ctx.enter_context(tc.tile_pool(name="psA", bufs=2, space="PSUM"))
        psB = ctx.enter_context(tc.tile_pool(name="psB", bufs=2, space="PSUM"))

        xT_sb = const.tile([128, KC, N], BF16)
        wqk_sb = const.tile([128, KC, 2 * D], BF16)
        wv_sb = const.tile([128, KC, D], BF16)
        wp_sb = const.tile([128, KC, D], BF16)
        bias_sb = const.tile([128, D], F32)
        qk_sb = const.tile([128, 2 * KC, N], BF16)   # chunks 0-5: qT, 6-11: kT
        v_sb = const.tile([128, NT, H * VW], BF16)
        attn_sb = const.tile([128, KC, N], BF16)     # attn_out^T, normalized

        for c in range(KC):
            nc.sync.dma_start(out=xT_sb[:, c, :], in_=xT[128 * c:128 * (c + 1), :])
        for c in range(KC):
            nc.sync.dma_start(out=wqk_sb[:, c, :], in_=w_qk[128 * c:128 * (c + 1), :])
            nc.sync.dma_start(out=wv_sb[:, c, :], in_=w_v[128 * c:128 * (c + 1), :])
            nc.sync.dma_start(out=wp_sb[:, c, :], in_=w_p[128 * c:128 * (c + 1), :])
        bap = bias[:, :]
        bias_bcast = bass.AP(
            tensor=bap.tensor, offset=bap.offset,
            ap=[[0, 128]] + list(bap.ap)[1:],
        )
        nc.gpsimd.dma_start(out=bias_sb, in_=bias_bcast)

        # Per-head v' weights [128 rows of keys, 128 cols]: col 0 = ones
        # (row-sum accumulator -> PSUM partition 0), cols 64..127 = v data
        # (-> PSUM partitions 64..127). Cols 1..63 are zero.
        v4 = v_sb.rearrange("p t (h e) -> p t h e", e=VW)
        nc.vector.memset(v_sb, 0.0)
        nc.vector.memset(v4[:, :, :, 0:1], 1.0)

        def emit_qkT(t):
            ps_qk = psA.tile([128, N], F32, tag="ps", name="ps_qk")
            for c in range(KC):
                for s in range(2):
                    nc.tensor.matmul(
                        ps_qk[:, 512 * s:512 * (s + 1)],
                        lhsT=wqk_sb[:, c, 128 * t:128 * (t + 1)],
                        rhs=xT_sb[:, c, 512 * s:512 * (s + 1)],
                        start=(c == 0), stop=(c == KC - 1),
                    )
            nc.vector.tensor_copy(out=qk_sb[:, t, :], in_=ps_qk)

        def emit_v(mt):
            ps_v = psB.tile([128, N], F32, tag="ps", name="ps_v")
            for c in range(KC):
                for lo, sz in ((0, 512), (512, 256)):
                    nc.tensor.matmul(
                        ps_v[:, lo:lo + sz],
                        lhsT=xT_sb[:, c, 128 * mt:128 * (mt + 1)],
                        rhs=wv_sb[:, c, lo:lo + sz],
                        start=(c == 0), stop=(c == KC - 1),
                    )
            nc.vector.tensor_copy(
                out=v4[:, mt, :, 64:128],
                in_=ps_v[:, 0:D].rearrange("p (h e) -> p h e", e=HD),
            )

        def emit_head_pair(p):
            tq, tk = p, KC + p
            ps_os = [
                psB.tile([128, N], F32, tag="ps", name="ps_o0"),
                psB.tile([128, N], F32, tag="ps", name="ps_o1"),
            ]
            for mt in range(NT):
                for par in range(2):
                    h = 2 * p + par
                    po = par * 64
                    ps_s = psA.tile([128, N], F32, tag="ps", name="ps_s")
                    for s in range(2):
                        nc.tensor.matmul(
                            ps_s[:, 512 * s:512 * (s + 1)],
                            lhsT=qk_sb[po:po + 64, tk, 128 * mt:128 * (mt + 1)],
                            rhs=qk_sb[po:po + 64, tq, 512 * s:512 * (s + 1)],
                            start=True, stop=True,
                        )
                    pt = work.tile([128, N], BF16, tag="pt", name="pt", bufs=4)
                    nc.scalar.activation(
                        out=pt, in_=ps_s,
                        func=mybir.ActivationFunctionType.Exp, scale=SCALE,
                    )
                    for s in range(2):
                        nc.tensor.matmul(
                            ps_os[par][:, 512 * s:512 * (s + 1)],
                            lhsT=v_sb[:, mt, VW * h:VW * (h + 1)],
                            rhs=pt[:, 512 * s:512 * (s + 1)],
                            start=(mt == 0), stop=(mt == NT - 1),
                        )
            for par in range(2):
                ps_o = ps_os[par]
                po = par * 64
                # sums on PSUM partition 0; v data on partitions 64..127.
                # (partition_broadcast/reciprocal_approx_fast only read from
                # base partition 0 on HW; DVE ops can't shift partitions.)
                recip = work.tile([1, N], F32, tag="recip", name="recip")
                nc.vector.reciprocal_approx_fast(out=recip, in_=ps_o[0:1, :])
                rb = work.tile([128, N], F32, tag="rb", name="rb")
                nc.gpsimd.partition_broadcast(rb, recip)
                tmp = work.tile([128, N], BF16, tag="tmp", name="tmp")
                nc.vector.tensor_mul(
                    out=tmp[64:128, :], in0=ps_o[64:128, :], in1=rb[64:128, :],
                )
                nc.sync.dma_start(
                    out=attn_sb[po:po + 64, tq, :], in_=tmp[64:128, :],
                )

        def emit_proj(nt):
            ps_p = psB.tile([128, N], F32, tag="ps", name="ps_p")
            for c in range(KC):
                for lo, sz in ((0, 512), (512, 256)):
                    nc.tensor.matmul(
                        ps_p[:, lo:lo + sz],
                        lhsT=attn_sb[:, c, 128 * nt:128 * (nt + 1)],
                        rhs=wp_sb[:, c, lo:lo + sz],
                        start=(c == 0), stop=(c == KC - 1),
                    )
            o_sb = work.tile([128, D], F32, tag="o_sb", name="o_sb")
            nc.vector.tensor_add(out=o_sb, in0=ps_p[:, 0:D], in1=bias_sb)
            nc.sync.dma_start(out=out[128 * nt:128 * (nt + 1), :], in_=o_sb)

        emit_qkT(0)
        emit_qkT(KC)
        for mt in range(NT):
            emit_v(mt)
        for p in range(KC):
            if p + 1 < KC:
                emit_qkT(p + 1)
                emit_qkT(KC + p + 1)
            emit_head_pair(p)
        for nt in range(NT):
            emit_proj(nt)

    nc.compile()
    return nc


def _get_nc():
    if "nc" not in _CACHE:
        _CACHE["nc"] = _build_nc()
    return _CACHE["nc"]


def _make_in_maps(x, W_qkv, W_proj, b_proj):
    bf = ml_dtypes.bfloat16
    x = np.asarray(x, dtype=np.float32)
    W_qkv = np.asarray(W_qkv, dtype=np.float32)
    W_proj = np.asarray(W_proj, dtype=np.float32)
    b_proj = np.asarray(b_proj, dtype=np.float32)
    w_qk = np.ascontiguousarray(W_qkv[:, :2 * D]).astype(bf)
    w_v = np.ascontiguousarray(W_qkv[:, 2 * D:]).astype(bf)
    w_p = W_proj.astype(bf)
    bias = b_proj.reshape(1, D)
    return [
        {
            "xT": np.ascontiguousarray(x[b].T).astype(bf),
            "w_qk": w_qk,
            "w_v": w_v,
            "w_p": w_p,
            "bias": bias,
        }
        for b in range(NCORES)
    ]


def run(x, W_qkv, W_proj, b_proj, trace=False):
    nc = _get_nc()
    in_maps = _make_in_maps(x, W_qkv, W_proj, b_proj)
    res = run_bass_kernel_spmd(nc, in_maps, core_ids=list(range(NCORES)), trace=trace)
    out = np.stack([res.results[b]["out"] for b in range(NCORES)], axis=0)
    return out.astype(np.float32), res


def kernel(x, W_qkv, W_proj, b_proj):
    out, _ = run(x, W_qkv, W_proj, b_proj, trace=False)
    return out
